# revision 46
# baseline (speedup 1.0000x reference)
"""Trainium2 Bass kernel for nn_BitwiseNetwork (STFT-style masking net).

Strategy (per core, data-parallel over batch, 4 batch items per core):
  - conv1 (1->1026ch, K=1024, S=256) as PE matmuls over a 128-sample "pieces"
    layout of x; stride-2 column APs give every frame without data duplication.
    in_gamma is folded into the conv1 weights on the host; tanh on ACT.
  - channels are permuted so real/imag cut pairs share partitions:
    tiles 0..7 each hold 4x(16 real + 16 imag) cuts; tile 8 = [r512, i512].
  - combine1 (2->8 per cut) as packed K=32 PE matmuls (block-diag weights),
    relu on ACT; combine2 (8->1) as K=128 scatter matmuls accumulated in PSUM.
  - linear 513x513 over cuts with output columns duplicated/interleaved so the
    mask tiles align 1:1 with the t tiles; mask = sigmoid(2g z + 2g b) 1 ACT op.
  - softmax(2ch) folded: out0 = sigmoid(convT_{w0-w1}(t*mask)), out1 = 1-out0.
    convT evaluated as 4 shifted PSUM-accumulated matmuls (frame overlap).
All matmuls run in float32r (~1e-4 rel err, ~4x faster than fp32 on PE).
"""

import numpy as np

import concourse.bass as bass
import concourse.bacc as bacc
import concourse.mybir as mybir
import concourse.tile as tile
from concourse.bass_utils import run_bass_kernel_spmd

f32 = mybir.dt.float32
f32r = mybir.dt.float32r
bf16 = mybir.dt.bfloat16
AF = mybir.ActivationFunctionType

KSZ, STR, CUT, TC, CH = 1024, 256, 513, 1026, 8
N_CORES = 8


def _chunks(total, maxc):
    n = (total + maxc - 1) // maxc
    base, rem = divmod(total, n)
    out, pos = [], 0
    for i in range(n):
        c = base + (1 if i < rem else 0)
        out.append((pos, c))
        pos += c
    return out


def _perm():
    # tile j partition p: p < 64 -> real cut 64j+p ; p >= 64 -> imag cut
    # 64j+(p-64). r/i in aligned 64-blocks so a compact mask tile applies
    # to both halves with plain partition-sliced DVE multiplies.
    p = []
    for j in range(8):
        p += [64 * j + c for c in range(64)]
        p += [513 + 64 * j + c for c in range(64)]
    p += [512, 513 + 512]
    return np.array(p)


def pack_weights(conv1_w, in_gamma, comb1_w, comb1_b, comb2_w, comb2_b,
                 lin_w, lin_b, fc_gamma, convT_w):
    perm = _perm()
    W1 = (np.asarray(conv1_w)[:, 0, :] * np.asarray(in_gamma)[:, None])[perm]
    wd = (np.asarray(convT_w)[:, 0, :] - np.asarray(convT_w)[:, 1, :])[perm]
    comb1_w = np.asarray(comb1_w); comb1_b = np.asarray(comb1_b)
    comb2_w = np.asarray(comb2_w); comb2_b = np.asarray(comb2_b)
    lin_w = np.asarray(lin_w); lin_b = np.asarray(lin_b)
    fc_gamma = np.asarray(fc_gamma)

    w1 = np.zeros((128, 8 * 8 * 128), np.float32)
    for i in range(8):
        for c in range(8):
            # lhsT[kk, m] = W1[128*i + m, 128*c + kk]
            w1[:, (i * 8 + c) * 128:(i * 8 + c + 1) * 128] = \
                W1[128 * i:128 * i + 128, 128 * c:128 * c + 128].T

    # combine1: per tile j, per hidden-pair hp (hid 2hp, 2hp+1), one K=128
    # matmul: out[m = 64u + c] = w(2hp+u, ri) * t[ri-half, cut 64j+c].
    c1 = np.zeros((128, 512), np.float32)
    for hp in range(4):
        for u in range(2):
            for c in range(64):
                c1[c, 128 * hp + 64 * u + c] = comb1_w[2 * hp + u, 0]
                c1[64 + c, 128 * hp + 64 * u + c] = comb1_w[2 * hp + u, 1]
    c1b = np.zeros((128, 4), np.float32)
    for hp in range(4):
        for u in range(2):
            c1b[64 * u:64 * u + 64, hp] = comb1_b[2 * hp + u]
    # combine2: accumulate hg tiles (j = 2a+parity, hp) into compact cut
    # tile a: out[m'] = cut 128a+m'; m' = 64*parity + c.
    v2 = np.zeros((128, 8 * 128), np.float32)
    for hp in range(4):
        for parity in range(2):
            g = 2 * hp + parity
            for u in range(2):
                for c in range(64):
                    v2[64 * u + c, 128 * g + 64 * parity + c] = \
                        comb2_w[0, 2 * hp + u]
    c2b = np.full((128, 1), comb2_b[0], np.float32)

    # linear over compact cut tiles: z[128jt+m] = sum_a lin_w block + h8 term
    lina = np.zeros((128, 4 * 512), np.float32)
    for a in range(4):
        for jt in range(4):
            lina[:, a * 512 + 128 * jt:a * 512 + 128 * jt + 128] = \
                lin_w[128 * jt:128 * jt + 128, 128 * a:128 * a + 128].T
    linrem = np.zeros((128, 8), np.float32)                # cut-512 row, M=2
    for a in range(4):
        linrem[:, 2 * a:2 * a + 2] = \
            np.repeat(lin_w[512:513, 128 * a:128 * a + 128].T, 2, axis=1)
    linb = np.zeros((1, 512), np.float32)
    for jt in range(4):
        linb[0, 128 * jt:128 * jt + 128] = lin_w[128 * jt:128 * jt + 128, 512]
    linb8 = np.full((1, 2), lin_w[512, 512], np.float32)

    msc = np.zeros((128, 4), np.float32)
    mbi = np.zeros((128, 4), np.float32)
    for jt in range(4):
        cs = np.arange(128 * jt, 128 * jt + 128)
        msc[:, jt] = 2.0 * fc_gamma[cs]
        mbi[:, jt] = 2.0 * fc_gamma[cs] * lin_b[cs]
    msc8 = np.full((2, 1), 2.0 * fc_gamma[512], np.float32)
    mbi8 = np.full((2, 1), 2.0 * fc_gamma[512] * lin_b[512], np.float32)

    # r/i mask duplication: psd[p, f] = mk[hi + p%64, f] via 0/1 matmul
    # (TensorTensor on HW needs equal SBUF base partitions, so the dup goes
    # through PE into PSUM instead).
    dup = np.zeros((128, 256), np.float32)
    for hi in range(2):
        for m in range(128):
            dup[64 * hi + (m % 64), 128 * hi + m] = 1.0

    # convT weights for q-partition output: out[q', sig] accumulates
    # lhsT = t-tile column slice (stationary), rhs = wdr[c, slice].
    # Odd frame shifts (jj=2,4) read a 1-shifted bf16 copy of masked t so
    # every stationary load starts at an even column (odd starts are slow).
    import ml_dtypes
    wdr = np.zeros((128, 8 * 512), np.float32)        # jj in {1, 3}
    wdrb = np.zeros((128, 8 * 512), np.float32)       # jj in {2, 4}
    for i in range(8):
        for idx, jj in enumerate((1, 3)):
            wdr[:, i * 512 + idx * 256:i * 512 + idx * 256 + 256] = \
                wd[128 * i:128 * i + 128, (4 - jj) * 256:(4 - jj) * 256 + 256]
        for idx, jj in enumerate((2, 4)):
            wdrb[:, i * 512 + idx * 256:i * 512 + idx * 256 + 256] = \
                wd[128 * i:128 * i + 128, (4 - jj) * 256:(4 - jj) * 256 + 256]
    wdrb = wdrb.astype(ml_dtypes.bfloat16)
    wdr8 = np.zeros((2, 1024), np.float32)
    for jj in range(1, 5):
        wdr8[:, (jj - 1) * 256:(jj - 1) * 256 + 256] = \
            wd[1024:1026, (4 - jj) * 256:(4 - jj) * 256 + 256]

    return dict(w1=w1, c1=c1, c1b=c1b,
                v2=v2, c2b=c2b, lina=lina, linrem=linrem, linb=linb,
                linb8=linb8, msc=msc, mbi=mbi, msc8=msc8, mbi8=mbi8,
                dup=dup, wdr=wdr, wdrb=wdrb, wdr8=wdr8,
                zpad=np.zeros((128, 10), np.float32))


_W_SHAPES = dict(w1=(128, 8192), c1=(128, 512),
                 c1b=(128, 4), v2=(128, 1024),
                 c2b=(128, 1), lina=(128, 2048),
                 linrem=(128, 8), linb=(1, 512), linb8=(1, 2),
                 msc=(128, 4), mbi=(128, 4), msc8=(2, 1),
                 mbi8=(2, 1), dup=(128, 256), wdr=(128, 4096),
                 wdrb=(128, 4096), wdr8=(2, 1024), zpad=(128, 10))
_F32R_W = {"w1", "c1", "v2", "lina", "linrem", "linb",
           "linb8", "dup", "wdr", "wdr8", "zpad"}
_BF16_W = {"wdrb"}


def host_t8_h8(xs, conv1_w, in_gamma, comb1_w, comb1_b, comb2_w, comb2_b,
               NFP):
    """Cut-512 path on host: t8 (2, NFP) and h8 (1, NFP) per batch item.

    xs: (BLOC, T) float32. Returns (t8, h8) as float32 arrays.
    """
    BLOC, T = xs.shape
    w2 = (np.asarray(conv1_w)[[512, 1025], 0, :]
          * np.asarray(in_gamma)[[512, 1025], None])          # (2, K)
    xp = np.pad(xs, ((0, 0), (KSZ, KSZ)))
    s = xp.strides
    frames = np.lib.stride_tricks.as_strided(
        xp[:, STR:], (BLOC, NFP, KSZ), (s[0], STR * s[1], s[1]))
    t8 = np.tanh(np.einsum("bfk,ck->bcf", frames, w2,
                           optimize=True)).astype(np.float32)
    u = np.maximum(np.einsum("oc,bcf->bof", np.asarray(comb1_w), t8)
                   + np.asarray(comb1_b)[None, :, None], 0.0)
    h8 = np.maximum(np.einsum("o,bof->bf", np.asarray(comb2_w)[0], u)
                    + np.asarray(comb2_b)[0], 0.0)
    return t8, h8[:, None, :].astype(np.float32)


def build_nc(T, BLOC, fch=344, t_bufs=9, h_bufs=8, loop_reps=1,
             convt_interleave=False, conv1_lookahead=False, nchains=2,
             skip_combine=False, skip_linear=False, skip_apply=False,
             skip_convt=False):
    P = T // 128
    NF = P // 2 + 3                 # frames used by convT: f = 1..NF
    NFP = NF + (NF & 1)             # padded even (fp32r needs even counts)
    NQ = T // 256
    fchunks = [(2 * c0, 2 * n) for (c0, n) in _chunks(NFP // 2, fch // 2)]

    nc = bacc.Bacc("TRN2", target_bir_lowering=False, debug=False,
                   num_devices=N_CORES)
    x_ap = nc.dram_tensor("x", (BLOC, 128, P), f32, kind="ExternalInput").ap()
    t8_ap = nc.dram_tensor("t8in", (BLOC, 2, NFP), f32,
                           kind="ExternalInput").ap()
    h8_ap = nc.dram_tensor("h8in", (BLOC, 1, NFP), f32,
                           kind="ExternalInput").ap()
    y_ap = nc.dram_tensor("y", (BLOC, 2, NQ, 256), f32,
                          kind="ExternalOutput").ap()
    w_aps = {k: nc.dram_tensor(k, s, bf16 if k in _BF16_W else f32,
                               kind="ExternalInput").ap()
             for k, s in _W_SHAPES.items()}

    with tile.TileContext(nc) as tc:
        with (tc.tile_pool(name="wpool", bufs=1) as wpool,
              tc.tile_pool(name="x2pool", bufs=2) as x2pool,
              tc.tile_pool(name="tpool", bufs=t_bufs) as tpool,
              tc.tile_pool(name="ttspool", bufs=t_bufs) as ttspool,
              tc.tile_pool(name="t8pool", bufs=2) as t8pool,
              tc.tile_pool(name="hpool", bufs=h_bufs) as hpool,
              tc.tile_pool(name="h8pool", bufs=2) as h8pool,
              tc.tile_pool(name="hgpool", bufs=6) as hgpool,
              tc.tile_pool(name="mpool", bufs=6) as mpool,
              tc.tile_pool(name="m8pool", bufs=2) as m8pool,
              tc.tile_pool(name="opool", bufs=4) as opool,
              tc.tile_pool(name="ps2pool", bufs=nchains,
                           space="PSUM") as ps2pool,
              tc.tile_pool(name="pspool", bufs=8 - nchains,
                           space="PSUM") as pspool):

            wsb = {}
            for k, shp in _W_SHAPES.items():
                dt = (bf16 if k in _BF16_W
                      else f32r if k in _F32R_W else f32)
                wt = wpool.tile(list(shp), dt, name=f"w_{k}")
                src = w_aps[k][:]
                if k in _F32R_W:
                    src = src.bitcast(f32r)
                nc.sync.dma_start(wt[:], src)
                wsb[k] = wt

            def ps_tile(name):
                return pspool.tile([128, 512], f32, tag="ps", name=name)

            def load_inputs(b):
                X2 = x2pool.tile([128, P + 18], f32r, tag="x2",
                                 name=f"X2_{b}")
                nc.sync.dma_start(X2[:, 0:8],
                                  w_aps["zpad"][:, 0:8].bitcast(f32r))
                nc.sync.dma_start(X2[:, 8 + P:],
                                  w_aps["zpad"][:].bitcast(f32r))
                nc.sync.dma_start(X2[:, 8:8 + P], x_ap[b].bitcast(f32r))
                t8 = t8pool.tile([2, NFP], f32r, tag="t8", name=f"t8_{b}")
                nc.sync.dma_start(t8[:], t8_ap[b].bitcast(f32r))
                h8 = h8pool.tile([1, NFP], f32r, tag="h8", name=f"h8_{b}")
                nc.sync.dma_start(h8[:], h8_ap[b].bitcast(f32r))
                return X2, t8, h8

            def emit_conv1(b, X2, tt, c0, ncols, irange=range(8)):
                f0 = c0 + 1
                for i in irange:
                    ps = ps_tile(f"psc_{b}_{i}_{c0}")
                    for c in range(8):
                        lhsT = wsb["w1"][:, (i * 8 + c) * 128:
                                         (i * 8 + c + 1) * 128]
                        rhs = X2[:, 2 * f0 + c: 2 * f0 + c + 2 * ncols: 2]
                        nc.tensor.matmul(ps[:128, :ncols], lhsT, rhs,
                                         start=(c == 0), stop=(c == 7))
                    nc.scalar.activation(tt[i][:, c0:c0 + ncols],
                                         ps[:128, :ncols], AF.Tanh)

            def emit_combine(b, tt, c0, ncols, apair, hh):
                # Phase A: all combine1 matmuls + relus into hg tiles.
                # Phase B: all combine2 accumulations back-to-back — no
                # PE<->ACT ping-pong inside the accumulation chain.
                ps2 = {a: ps2pool.tile([128, 512], f32, tag="ps2",
                                       name=f"ps2_{b}_{a}_{c0}")
                       for a in apair}
                k = 0
                for g in range(8):
                    hp, parity = g >> 1, g & 1
                    for a in apair:
                        j = 2 * a + parity
                        ps1 = ps_tile(f"ps1_{b}_{a}_{g}_{c0}")
                        nc.tensor.matmul(
                            ps1[:128, :ncols],
                            wsb["c1"][:, 128 * hp:128 * hp + 128],
                            tt[j][:, c0:c0 + ncols],
                            start=True, stop=True)
                        hg = hgpool.tile([128, fch], f32r, tag="hg",
                                         name=f"hg_{b}_{a}_{g}_{c0}")
                        if k % 2 == 0:
                            nc.scalar.activation(
                                hg[:, :ncols], ps1[:128, :ncols], AF.Relu,
                                bias=wsb["c1b"][:, hp:hp + 1])
                        else:
                            nc.vector.tensor_scalar(
                                hg[:, :ncols], ps1[:128, :ncols],
                                wsb["c1b"][:, hp:hp + 1], 0.0,
                                mybir.AluOpType.add,
                                mybir.AluOpType.max)
                        nc.tensor.matmul(
                            ps2[a][:128, :ncols],
                            wsb["v2"][:, 128 * g:128 * g + 128],
                            hg[:, :ncols],
                            start=(g == 0), stop=(g == 7))
                        k += 1
                for a in apair:
                    nc.scalar.activation(hh[a][:, :ncols],
                                         ps2[a][:128, :ncols], AF.Relu,
                                         bias=wsb["c2b"][:, 0:1])

            def emit_linear_apply(b, tt, t8, h8, hh, c0, ncols):
                # compact linear + mask; apply dups r/i via partition-sliced
                # DVE multiplies.
                mks = []
                for jt in range(4):
                    ps3 = ps_tile(f"ps3_{b}_{jt}_{c0}")
                    for a in range(4):
                        nc.tensor.matmul(
                            ps3[:128, :ncols],
                            wsb["lina"][:, a * 512 + 128 * jt:
                                        a * 512 + 128 * jt + 128],
                            hh[a][:, :ncols],
                            start=(a == 0), stop=False)
                    nc.tensor.matmul(
                        ps3[:128, :ncols],
                        wsb["linb"][0:1, 128 * jt:128 * jt + 128],
                        h8[0:1, c0:c0 + ncols],
                        start=False, stop=True)
                    mk = mpool.tile([128, fch], f32r, tag="m",
                                    name=f"mk_{b}_{jt}_{c0}")
                    nc.scalar.activation(
                        mk[:, :ncols], ps3[:128, :ncols], AF.Sigmoid,
                        bias=wsb["mbi"][:, jt:jt + 1],
                        scale=wsb["msc"][:, jt:jt + 1])
                    mks.append(mk)
                ps3r = ps_tile(f"ps3r_{b}_{c0}")
                for a in range(4):
                    nc.tensor.matmul(ps3r[:2, :ncols],
                                     wsb["linrem"][:, 2 * a:2 * a + 2],
                                     hh[a][:, :ncols],
                                     start=(a == 0), stop=False)
                nc.tensor.matmul(ps3r[:2, :ncols],
                                 wsb["linb8"][0:1, 0:2],
                                 h8[0:1, c0:c0 + ncols],
                                 start=False, stop=True)
                mk8 = m8pool.tile([2, fch], f32r, tag="m8",
                                  name=f"mk8_{b}_{c0}")
                nc.scalar.activation(
                    mk8[:, :ncols], ps3r[:2, :ncols], AF.Sigmoid,
                    bias=wsb["mbi8"][:, 0:1],
                    scale=wsb["msc8"][:, 0:1])
                if skip_apply:
                    return
                for j in range(8):
                    a, par = j // 2, j % 2
                    psd = ps_tile(f"psd_{b}_{j}_{c0}")
                    nc.tensor.matmul(psd[:128, :ncols],
                                     wsb["dup"][:, 128 * par:128 * par + 128],
                                     mks[a][:, :ncols],
                                     start=True, stop=True)
                    nc.vector.tensor_mul(
                        tt[j][:, c0:c0 + ncols],
                        tt[j][:, c0:c0 + ncols],
                        psd[:128, :ncols])
                nc.vector.tensor_mul(t8[:, c0:c0 + ncols],
                                     t8[:, c0:c0 + ncols],
                                     mk8[:, :ncols])

            def emit_convT(b, tt, tts, t8, qb):
                # out[q', sig] per q-block: partitions = frame index, so the
                # y store is fully contiguous in DRAM (no transposed DMA).
                # Even frame shifts read tt (f32r); odd shifts read the
                # 1-shifted bf16 copy tts so stationary loads stay aligned.
                ps4 = ps_tile(f"ps4_{b}_{qb}")
                idx = 0
                for jj in range(1, 5):
                    for i in range(9):
                        base = 128 * qb + jj - 1
                        if i < 8:
                            if jj % 2 == 1:
                                lhsT = tt[i][:, base:base + 128]
                                w, ix = "wdr", (jj - 1) // 2
                            else:
                                lhsT = tts[i][:, base - 1:base - 1 + 128]
                                w, ix = "wdrb", (jj - 2) // 2
                            rhs = wsb[w][:, i * 512 + ix * 256:
                                         i * 512 + ix * 256 + 256]
                        else:
                            lhsT = t8[:, base:base + 128]
                            rhs = wsb["wdr8"][:, (jj - 1) * 256:
                                              (jj - 1) * 256 + 256]
                        nc.tensor.matmul(ps4[:128, :256], lhsT, rhs,
                                         start=(idx == 0), stop=(idx == 35))
                        idx += 1
                o0 = opool.tile([128, 256], f32, tag="o", name=f"o0_{b}_{qb}")
                nc.scalar.activation(o0[:, :256], ps4[:128, :256], AF.Sigmoid)
                o1 = opool.tile([128, 256], f32, tag="o", name=f"o1_{b}_{qb}")
                nc.vector.tensor_scalar(
                    o1[:, :256], o0[:, :256], -1.0, 1.0,
                    mybir.AluOpType.mult, mybir.AluOpType.add)
                nc.sync.dma_start(
                    y_ap[b, 0, 128 * qb:128 * qb + 128, :], o0[:, :256])
                nc.sync.dma_start(
                    y_ap[b, 1, 128 * qb:128 * qb + 128, :], o1[:, :256])

            def emit_batch(b, ins, prefetch):
                X2, t8, h8 = ins
                NQB = NQ // 128
                tt = [tpool.tile([128, NFP], f32r, tag="t", name=f"t{b}_{j}")
                      for j in range(8)]
                tts = [ttspool.tile([128, NFP], bf16, tag="ts",
                                    name=f"ts{b}_{j}") for j in range(8)]
                emit_conv1(b, X2, tt, *fchunks[0])
                if prefetch is not None:
                    prefetch()
                qb_done = 0
                for ci, (c0, ncols) in enumerate(fchunks):
                    if not conv1_lookahead and ci > 0:
                        emit_conv1(b, X2, tt, c0, ncols)
                    hh = [hpool.tile([128, fch], f32r, tag="h",
                                     name=f"h{b}_{a}_{c0}") for a in range(4)]
                    la = conv1_lookahead and ci + 1 < len(fchunks)
                    if skip_combine:
                        continue
                    if nchains == 4:
                        emit_combine(b, tt, c0, ncols, (0, 1, 2, 3), hh)
                        if la:
                            emit_conv1(b, X2, tt, *fchunks[ci + 1])
                    else:
                        emit_combine(b, tt, c0, ncols, (0, 1), hh)
                        if la:
                            emit_conv1(b, X2, tt, *fchunks[ci + 1],
                                       irange=range(4))
                        emit_combine(b, tt, c0, ncols, (2, 3), hh)
                        if la:
                            emit_conv1(b, X2, tt, *fchunks[ci + 1],
                                       irange=range(4, 8))
                    if not skip_linear:
                        emit_linear_apply(b, tt, t8, h8, hh, c0, ncols)
                    # convT q-blocks whose frame window is fully masked
                    if not skip_linear and not skip_combine:
                        s0 = 0 if ci == 0 else c0 - 1
                        e0 = c0 + ncols - 1 if ci + 1 < len(fchunks) \
                            else NFP - 2
                        for j in range(8):
                            if (j % 2) == 0:
                                nc.scalar.activation(
                                    tts[j][:, s0:e0],
                                    tt[j][:, s0 + 1:e0 + 1], AF.Copy)
                            else:
                                nc.vector.tensor_scalar(
                                    tts[j][:, s0:e0],
                                    tt[j][:, s0 + 1:e0 + 1], 1.0, None,
                                    mybir.AluOpType.mult)
                    last = ci == len(fchunks) - 1
                    while not skip_convt and qb_done < NQB and (
                            last or (convt_interleave
                                     and 128 * qb_done + 131 <= c0 + ncols)):
                        emit_convT(b, tt, tts, t8, qb_done)
                        qb_done += 1
                if skip_convt:
                    emit_convT(b, tt, tts, t8, 0)
                if skip_combine and not skip_convt:
                    for j in range(8):
                        nc.scalar.activation(tts[j][:, 0:NFP - 2],
                                             tt[j][:, 1:NFP - 1], AF.Copy)
                    for qb in range(NQB):
                        emit_convT(b, tt, tts, t8, qb)

            def emit_all():
                ins = load_inputs(0)
                nxt = {}
                for b in range(BLOC):
                    if b + 1 < BLOC:
                        def prefetch(b=b):
                            nxt["ins"] = load_inputs(b + 1)
                        emit_batch(b, ins, prefetch)
                        ins = nxt.pop("ins")
                    else:
                        emit_batch(b, ins, None)

            if loop_reps == 1:
                emit_all()
            else:
                with tc.For_i(0, loop_reps, 1):
                    emit_all()
    nc.compile()
    return nc


_NC_CACHE = {}


def _get_nc(T, BLOC):
    key = (T, BLOC)
    if key not in _NC_CACHE:
        _NC_CACHE[key] = build_nc(T, BLOC)
    return _NC_CACHE[key]


def make_in_maps(x, conv1_w, in_gamma, comb1_w, comb1_b, comb2_w, comb2_b,
                 lin_w, lin_b, fc_gamma, convT_w):
    x = np.asarray(x)
    B, _, T = x.shape
    BLOC = B // N_CORES
    P = T // 128
    NF = P // 2 + 3
    NFP = NF + (NF & 1)
    w = pack_weights(conv1_w, in_gamma, comb1_w, comb1_b, comb2_w, comb2_b,
                     lin_w, lin_b, fc_gamma, convT_w)
    in_maps = []
    for core in range(N_CORES):
        shard = x[core * BLOC:(core + 1) * BLOC, 0, :]
        xt = np.ascontiguousarray(
            shard.reshape(BLOC, P, 128).transpose(0, 2, 1))
        t8, h8 = host_t8_h8(np.ascontiguousarray(shard), conv1_w, in_gamma,
                            comb1_w, comb1_b, comb2_w, comb2_b, NFP)
        m = {"x": xt, "t8in": t8, "h8in": h8}
        m.update(w)
        in_maps.append(m)
    return in_maps


def kernel(x, conv1_w, in_gamma, comb1_w, comb1_b, comb2_w, comb2_b,
           lin_w, lin_b, fc_gamma, convT_w):
    x = np.asarray(x)
    B, _, T = x.shape
    BLOC = B // N_CORES
    nc = _get_nc(T, BLOC)
    in_maps = make_in_maps(x, conv1_w, in_gamma, comb1_w, comb1_b, comb2_w,
                           comb2_b, lin_w, lin_b, fc_gamma, convT_w)
    res = run_bass_kernel_spmd(nc, in_maps, core_ids=list(range(N_CORES)))
    outs = [r["y"].reshape(BLOC, 2, T) for r in res.results]
    return np.concatenate(outs, axis=0)



# revision 47
# speedup vs baseline: 1.2005x; 1.2005x over previous
"""Trainium2 Bass kernel for nn_BitwiseNetwork (STFT-style masking net).

Strategy (per core, data-parallel over batch, 4 batch items per core):
  - conv1 (1->1026ch, K=1024, S=256) as PE matmuls over a 128-sample "pieces"
    layout of x; stride-2 column APs give every frame without data duplication.
    in_gamma is folded into the conv1 weights on the host; tanh on ACT.
  - channels are permuted so real/imag cut pairs share partitions:
    tiles 0..7 each hold 4x(16 real + 16 imag) cuts; tile 8 = [r512, i512].
  - combine1 (2->8 per cut) as packed K=32 PE matmuls (block-diag weights),
    relu on ACT; combine2 (8->1) as K=128 scatter matmuls accumulated in PSUM.
  - linear 513x513 over cuts with output columns duplicated/interleaved so the
    mask tiles align 1:1 with the t tiles; mask = sigmoid(2g z + 2g b) 1 ACT op.
  - softmax(2ch) folded: out0 = sigmoid(convT_{w0-w1}(t*mask)), out1 = 1-out0.
    convT evaluated as 4 shifted PSUM-accumulated matmuls (frame overlap).
All matmuls run in float32r (~1e-4 rel err, ~4x faster than fp32 on PE).
"""

import numpy as np

import concourse.bass as bass
import concourse.bacc as bacc
import concourse.mybir as mybir
import concourse.tile as tile
from concourse.bass_utils import run_bass_kernel_spmd

f32 = mybir.dt.float32
f32r = mybir.dt.float32r
bf16 = mybir.dt.bfloat16
AF = mybir.ActivationFunctionType

KSZ, STR, CUT, TC, CH = 1024, 256, 513, 1026, 8
N_CORES = 8


def _chunks(total, maxc):
    n = (total + maxc - 1) // maxc
    base, rem = divmod(total, n)
    out, pos = [], 0
    for i in range(n):
        c = base + (1 if i < rem else 0)
        out.append((pos, c))
        pos += c
    return out


def _perm():
    # tile j partition p: p < 64 -> real cut 64j+p ; p >= 64 -> imag cut
    # 64j+(p-64). r/i in aligned 64-blocks so a compact mask tile applies
    # to both halves with plain partition-sliced DVE multiplies.
    p = []
    for j in range(8):
        p += [64 * j + c for c in range(64)]
        p += [513 + 64 * j + c for c in range(64)]
    p += [512, 513 + 512]
    return np.array(p)


def pack_weights(conv1_w, in_gamma, comb1_w, comb1_b, comb2_w, comb2_b,
                 lin_w, lin_b, fc_gamma, convT_w):
    perm = _perm()
    W1 = (np.asarray(conv1_w)[:, 0, :] * np.asarray(in_gamma)[:, None])[perm]
    wd = (np.asarray(convT_w)[:, 0, :] - np.asarray(convT_w)[:, 1, :])[perm]
    comb1_w = np.asarray(comb1_w); comb1_b = np.asarray(comb1_b)
    comb2_w = np.asarray(comb2_w); comb2_b = np.asarray(comb2_b)
    lin_w = np.asarray(lin_w); lin_b = np.asarray(lin_b)
    fc_gamma = np.asarray(fc_gamma)

    w1 = np.zeros((128, 8 * 8 * 128), np.float32)
    for i in range(8):
        for c in range(8):
            # lhsT[kk, m] = W1[128*i + m, 128*c + kk]
            w1[:, (i * 8 + c) * 128:(i * 8 + c + 1) * 128] = \
                W1[128 * i:128 * i + 128, 128 * c:128 * c + 128].T

    # combine1: per tile j, per hidden-pair hp (hid 2hp, 2hp+1), one K=128
    # matmul: out[m = 64u + c] = w(2hp+u, ri) * t[ri-half, cut 64j+c].
    c1 = np.zeros((128, 512), np.float32)
    for hp in range(4):
        for u in range(2):
            for c in range(64):
                c1[c, 128 * hp + 64 * u + c] = comb1_w[2 * hp + u, 0]
                c1[64 + c, 128 * hp + 64 * u + c] = comb1_w[2 * hp + u, 1]
    c1b = np.zeros((128, 4), np.float32)
    for hp in range(4):
        for u in range(2):
            c1b[64 * u:64 * u + 64, hp] = comb1_b[2 * hp + u]
    # combine2: accumulate hg tiles (j = 2a+parity, hp) into compact cut
    # tile a: out[m'] = cut 128a+m'; m' = 64*parity + c.
    v2 = np.zeros((128, 8 * 128), np.float32)
    for hp in range(4):
        for parity in range(2):
            g = 2 * hp + parity
            for u in range(2):
                for c in range(64):
                    v2[64 * u + c, 128 * g + 64 * parity + c] = \
                        comb2_w[0, 2 * hp + u]
    c2b = np.full((128, 1), comb2_b[0], np.float32)

    # linear over compact cut tiles: z[128jt+m] = sum_a lin_w block + h8 term
    lina = np.zeros((128, 4 * 512), np.float32)
    for a in range(4):
        for jt in range(4):
            lina[:, a * 512 + 128 * jt:a * 512 + 128 * jt + 128] = \
                lin_w[128 * jt:128 * jt + 128, 128 * a:128 * a + 128].T
    linrem = np.zeros((128, 8), np.float32)                # cut-512 row, M=2
    for a in range(4):
        linrem[:, 2 * a:2 * a + 2] = \
            np.repeat(lin_w[512:513, 128 * a:128 * a + 128].T, 2, axis=1)
    linb = np.zeros((1, 512), np.float32)
    for jt in range(4):
        linb[0, 128 * jt:128 * jt + 128] = lin_w[128 * jt:128 * jt + 128, 512]
    linb8 = np.full((1, 2), lin_w[512, 512], np.float32)

    msc = np.zeros((128, 4), np.float32)
    mbi = np.zeros((128, 4), np.float32)
    for jt in range(4):
        cs = np.arange(128 * jt, 128 * jt + 128)
        msc[:, jt] = 2.0 * fc_gamma[cs]
        mbi[:, jt] = 2.0 * fc_gamma[cs] * lin_b[cs]
    msc8 = np.full((2, 1), 2.0 * fc_gamma[512], np.float32)
    mbi8 = np.full((2, 1), 2.0 * fc_gamma[512] * lin_b[512], np.float32)

    # r/i mask duplication: psd[p, f] = mk[hi + p%64, f] via 0/1 matmul
    # (TensorTensor on HW needs equal SBUF base partitions, so the dup goes
    # through PE into PSUM instead).
    dup = np.zeros((128, 256), np.float32)
    for hi in range(2):
        for m in range(128):
            dup[64 * hi + (m % 64), 128 * hi + m] = 1.0

    # convT weights for q-partition output: out[q', sig] accumulates
    # lhsT = t-tile column slice (stationary), rhs = wdr[c, slice].
    # Odd frame shifts (jj=2,4) read a 1-shifted bf16 copy of masked t so
    # every stationary load starts at an even column (odd starts are slow).
    import ml_dtypes
    wdr = np.zeros((128, 8 * 512), np.float32)        # jj in {1, 3}
    wdrb = np.zeros((128, 8 * 512), np.float32)       # jj in {2, 4}
    for i in range(8):
        for idx, jj in enumerate((1, 3)):
            wdr[:, i * 512 + idx * 256:i * 512 + idx * 256 + 256] = \
                wd[128 * i:128 * i + 128, (4 - jj) * 256:(4 - jj) * 256 + 256]
        for idx, jj in enumerate((2, 4)):
            wdrb[:, i * 512 + idx * 256:i * 512 + idx * 256 + 256] = \
                wd[128 * i:128 * i + 128, (4 - jj) * 256:(4 - jj) * 256 + 256]
    wdrb = wdrb.astype(ml_dtypes.bfloat16)
    wdr8 = np.zeros((2, 1024), np.float32)
    for jj in range(1, 5):
        wdr8[:, (jj - 1) * 256:(jj - 1) * 256 + 256] = \
            wd[1024:1026, (4 - jj) * 256:(4 - jj) * 256 + 256]

    return dict(w1=w1, c1=c1, c1b=c1b,
                v2=v2, c2b=c2b, lina=lina, linrem=linrem, linb=linb,
                linb8=linb8, msc=msc, mbi=mbi, msc8=msc8, mbi8=mbi8,
                dup=dup, wdr=wdr, wdrb=wdrb, wdr8=wdr8,
                zpad=np.zeros((128, 10), np.float32))


_W_SHAPES = dict(w1=(128, 8192), c1=(128, 512),
                 c1b=(128, 4), v2=(128, 1024),
                 c2b=(128, 1), lina=(128, 2048),
                 linrem=(128, 8), linb=(1, 512), linb8=(1, 2),
                 msc=(128, 4), mbi=(128, 4), msc8=(2, 1),
                 mbi8=(2, 1), dup=(128, 256), wdr=(128, 4096),
                 wdrb=(128, 4096), wdr8=(2, 1024), zpad=(128, 10))
_F32R_W = {"w1", "c1", "v2", "lina", "linrem", "linb",
           "linb8", "dup", "wdr", "wdr8", "zpad"}
_BF16_W = {"wdrb"}


def host_t8_h8(xs, conv1_w, in_gamma, comb1_w, comb1_b, comb2_w, comb2_b,
               NFP):
    """Cut-512 path on host: t8 (2, NFP) and h8 (1, NFP) per batch item.

    xs: (BLOC, T) float32. Returns (t8, h8) as float32 arrays.
    """
    BLOC, T = xs.shape
    w2 = (np.asarray(conv1_w)[[512, 1025], 0, :]
          * np.asarray(in_gamma)[[512, 1025], None])          # (2, K)
    xp = np.pad(xs, ((0, 0), (KSZ, KSZ)))
    s = xp.strides
    frames = np.lib.stride_tricks.as_strided(
        xp[:, STR:], (BLOC, NFP, KSZ), (s[0], STR * s[1], s[1]))
    t8 = np.tanh(np.einsum("bfk,ck->bcf", frames, w2,
                           optimize=True)).astype(np.float32)
    u = np.maximum(np.einsum("oc,bcf->bof", np.asarray(comb1_w), t8)
                   + np.asarray(comb1_b)[None, :, None], 0.0)
    h8 = np.maximum(np.einsum("o,bof->bf", np.asarray(comb2_w)[0], u)
                    + np.asarray(comb2_b)[0], 0.0)
    return t8, h8[:, None, :].astype(np.float32)


def build_nc(T, BLOC, fch=344, t_bufs=9, h_bufs=8, loop_reps=1,
             convt_interleave=False, conv1_lookahead=False, nchains=2,
             skip_combine=False, skip_linear=False, skip_apply=False,
             skip_convt=False):
    P = T // 128
    NF = P // 2 + 3                 # frames used by convT: f = 1..NF
    NFP = NF + (NF & 1)             # padded even (fp32r needs even counts)
    NQ = T // 256
    fchunks = [(2 * c0, 2 * n) for (c0, n) in _chunks(NFP // 2, fch // 2)]

    nc = bacc.Bacc("TRN2", target_bir_lowering=False, debug=False,
                   num_devices=N_CORES)
    x_ap = nc.dram_tensor("x", (BLOC, 128, P), f32, kind="ExternalInput").ap()
    t8_ap = nc.dram_tensor("t8in", (BLOC, 2, NFP), f32,
                           kind="ExternalInput").ap()
    h8_ap = nc.dram_tensor("h8in", (BLOC, 1, NFP), f32,
                           kind="ExternalInput").ap()
    y_ap = nc.dram_tensor("y", (BLOC, 2, NQ, 256), f32,
                          kind="ExternalOutput").ap()
    w_aps = {k: nc.dram_tensor(k, s, bf16 if k in _BF16_W else f32,
                               kind="ExternalInput").ap()
             for k, s in _W_SHAPES.items()}

    with tile.TileContext(nc) as tc:
        with (tc.tile_pool(name="wpool", bufs=1) as wpool,
              tc.tile_pool(name="x2pool", bufs=2) as x2pool,
              tc.tile_pool(name="tpool", bufs=t_bufs) as tpool,
              tc.tile_pool(name="ttspool", bufs=t_bufs) as ttspool,
              tc.tile_pool(name="t8pool", bufs=2) as t8pool,
              tc.tile_pool(name="hpool", bufs=h_bufs) as hpool,
              tc.tile_pool(name="h8pool", bufs=2) as h8pool,
              tc.tile_pool(name="hgpool", bufs=6) as hgpool,
              tc.tile_pool(name="mpool", bufs=6) as mpool,
              tc.tile_pool(name="m8pool", bufs=2) as m8pool,
              tc.tile_pool(name="opool", bufs=4) as opool,
              tc.tile_pool(name="ps2pool", bufs=nchains,
                           space="PSUM") as ps2pool,
              tc.tile_pool(name="pspool", bufs=8 - nchains,
                           space="PSUM") as pspool):

            wsb = {}
            for k, shp in _W_SHAPES.items():
                dt = (bf16 if k in _BF16_W
                      else f32r if k in _F32R_W else f32)
                wt = wpool.tile(list(shp), dt, name=f"w_{k}")
                src = w_aps[k][:]
                if k in _F32R_W:
                    src = src.bitcast(f32r)
                nc.sync.dma_start(wt[:], src)
                wsb[k] = wt

            def ps_tile(name):
                return pspool.tile([128, 512], f32, tag="ps", name=name)

            def load_inputs(b):
                X2 = x2pool.tile([128, P + 18], f32r, tag="x2",
                                 name=f"X2_{b}")
                nc.sync.dma_start(X2[:, 0:8],
                                  w_aps["zpad"][:, 0:8].bitcast(f32r))
                nc.sync.dma_start(X2[:, 8 + P:],
                                  w_aps["zpad"][:].bitcast(f32r))
                nc.sync.dma_start(X2[:, 8:8 + P], x_ap[b].bitcast(f32r))
                t8 = t8pool.tile([2, NFP], f32r, tag="t8", name=f"t8_{b}")
                nc.sync.dma_start(t8[:], t8_ap[b].bitcast(f32r))
                h8 = h8pool.tile([1, NFP], f32r, tag="h8", name=f"h8_{b}")
                nc.sync.dma_start(h8[:], h8_ap[b].bitcast(f32r))
                return X2, t8, h8

            def emit_conv1(b, X2, tt, c0, ncols, irange=range(8)):
                f0 = c0 + 1
                for i in irange:
                    ps = ps_tile(f"psc_{b}_{i}_{c0}")
                    for c in range(8):
                        lhsT = wsb["w1"][:, (i * 8 + c) * 128:
                                         (i * 8 + c + 1) * 128]
                        rhs = X2[:, 2 * f0 + c: 2 * f0 + c + 2 * ncols: 2]
                        nc.tensor.matmul(ps[:128, :ncols], lhsT, rhs,
                                         start=(c == 0), stop=(c == 7))
                    nc.scalar.activation(tt[i][:, c0:c0 + ncols],
                                         ps[:128, :ncols], AF.Tanh)

            def emit_combine(b, tt, c0, ncols, apair, hh):
                # Phase A: all combine1 matmuls + relus into hg tiles.
                # Phase B: all combine2 accumulations back-to-back — no
                # PE<->ACT ping-pong inside the accumulation chain.
                ps2 = {a: ps2pool.tile([128, 512], f32, tag="ps2",
                                       name=f"ps2_{b}_{a}_{c0}")
                       for a in apair}
                k = 0
                for g in range(8):
                    hp, parity = g >> 1, g & 1
                    for a in apair:
                        j = 2 * a + parity
                        ps1 = ps_tile(f"ps1_{b}_{a}_{g}_{c0}")
                        nc.tensor.matmul(
                            ps1[:128, :ncols],
                            wsb["c1"][:, 128 * hp:128 * hp + 128],
                            tt[j][:, c0:c0 + ncols],
                            start=True, stop=True)
                        hg = hgpool.tile([128, fch], f32r, tag="hg",
                                         name=f"hg_{b}_{a}_{g}_{c0}")
                        if k % 2 == 0:
                            nc.scalar.activation(
                                hg[:, :ncols], ps1[:128, :ncols], AF.Relu,
                                bias=wsb["c1b"][:, hp:hp + 1])
                        else:
                            nc.vector.tensor_scalar(
                                hg[:, :ncols], ps1[:128, :ncols],
                                wsb["c1b"][:, hp:hp + 1], 0.0,
                                mybir.AluOpType.add,
                                mybir.AluOpType.max)
                        nc.tensor.matmul(
                            ps2[a][:128, :ncols],
                            wsb["v2"][:, 128 * g:128 * g + 128],
                            hg[:, :ncols],
                            start=(g == 0), stop=(g == 7))
                        k += 1
                for a in apair:
                    nc.scalar.activation(hh[a][:, :ncols],
                                         ps2[a][:128, :ncols], AF.Relu,
                                         bias=wsb["c2b"][:, 0:1])

            def emit_linear_apply(b, tt, t8, h8, hh, c0, ncols):
                # compact linear + mask; apply dups r/i via partition-sliced
                # DVE multiplies.
                mks = []
                for jt in range(4):
                    ps3 = ps_tile(f"ps3_{b}_{jt}_{c0}")
                    for a in range(4):
                        nc.tensor.matmul(
                            ps3[:128, :ncols],
                            wsb["lina"][:, a * 512 + 128 * jt:
                                        a * 512 + 128 * jt + 128],
                            hh[a][:, :ncols],
                            start=(a == 0), stop=False)
                    nc.tensor.matmul(
                        ps3[:128, :ncols],
                        wsb["linb"][0:1, 128 * jt:128 * jt + 128],
                        h8[0:1, c0:c0 + ncols],
                        start=False, stop=True)
                    mk = mpool.tile([128, fch], f32r, tag="m",
                                    name=f"mk_{b}_{jt}_{c0}")
                    nc.scalar.activation(
                        mk[:, :ncols], ps3[:128, :ncols], AF.Sigmoid,
                        bias=wsb["mbi"][:, jt:jt + 1],
                        scale=wsb["msc"][:, jt:jt + 1])
                    mks.append(mk)
                ps3r = ps_tile(f"ps3r_{b}_{c0}")
                for a in range(4):
                    nc.tensor.matmul(ps3r[:2, :ncols],
                                     wsb["linrem"][:, 2 * a:2 * a + 2],
                                     hh[a][:, :ncols],
                                     start=(a == 0), stop=False)
                nc.tensor.matmul(ps3r[:2, :ncols],
                                 wsb["linb8"][0:1, 0:2],
                                 h8[0:1, c0:c0 + ncols],
                                 start=False, stop=True)
                mk8 = m8pool.tile([2, fch], f32r, tag="m8",
                                  name=f"mk8_{b}_{c0}")
                nc.scalar.activation(
                    mk8[:, :ncols], ps3r[:2, :ncols], AF.Sigmoid,
                    bias=wsb["mbi8"][:, 0:1],
                    scale=wsb["msc8"][:, 0:1])
                if skip_apply:
                    return
                for j in range(8):
                    a, par = j // 2, j % 2
                    psd = ps_tile(f"psd_{b}_{j}_{c0}")
                    nc.tensor.matmul(psd[:128, :ncols],
                                     wsb["dup"][:, 128 * par:128 * par + 128],
                                     mks[a][:, :ncols],
                                     start=True, stop=True)
                    nc.vector.tensor_mul(
                        tt[j][:, c0:c0 + ncols],
                        tt[j][:, c0:c0 + ncols],
                        psd[:128, :ncols])
                nc.vector.tensor_mul(t8[:, c0:c0 + ncols],
                                     t8[:, c0:c0 + ncols],
                                     mk8[:, :ncols])

            def emit_convT(b, tt, tts, t8, qb):
                # out[q', sig] per q-block: partitions = frame index, so the
                # y store is fully contiguous in DRAM (no transposed DMA).
                # Even frame shifts read tt (f32r); odd shifts read the
                # 1-shifted bf16 copy tts so stationary loads stay aligned.
                ps4 = ps_tile(f"ps4_{b}_{qb}")
                idx = 0
                for jj in range(1, 5):
                    for i in range(9):
                        base = 128 * qb + jj - 1
                        if i < 8:
                            if jj % 2 == 1:
                                lhsT = tt[i][:, base:base + 128]
                                w, ix = "wdr", (jj - 1) // 2
                            else:
                                lhsT = tts[i][:, base - 1:base - 1 + 128]
                                w, ix = "wdrb", (jj - 2) // 2
                            rhs = wsb[w][:, i * 512 + ix * 256:
                                         i * 512 + ix * 256 + 256]
                        else:
                            lhsT = t8[:, base:base + 128]
                            rhs = wsb["wdr8"][:, (jj - 1) * 256:
                                              (jj - 1) * 256 + 256]
                        nc.tensor.matmul(ps4[:128, :256], lhsT, rhs,
                                         start=(idx == 0), stop=(idx == 35))
                        idx += 1
                o0 = opool.tile([128, 256], f32, tag="o", name=f"o0_{b}_{qb}")
                nc.scalar.activation(o0[:, :256], ps4[:128, :256], AF.Sigmoid)
                o1 = opool.tile([128, 256], f32, tag="o", name=f"o1_{b}_{qb}")
                nc.vector.tensor_scalar(
                    o1[:, :256], o0[:, :256], -1.0, 1.0,
                    mybir.AluOpType.mult, mybir.AluOpType.add)
                nc.sync.dma_start(
                    y_ap[b, 0, 128 * qb:128 * qb + 128, :], o0[:, :256])
                nc.sync.dma_start(
                    y_ap[b, 1, 128 * qb:128 * qb + 128, :], o1[:, :256])

            def emit_batch(b, ins, prefetch):
                X2, t8, h8 = ins
                NQB = NQ // 128
                tt = [tpool.tile([128, NFP], f32r, tag="t", name=f"t{b}_{j}")
                      for j in range(8)]
                tts = [ttspool.tile([128, NFP], bf16, tag="ts",
                                    name=f"ts{b}_{j}") for j in range(8)]
                emit_conv1(b, X2, tt, *fchunks[0])
                if prefetch is not None:
                    prefetch()
                qb_done = 0
                for ci, (c0, ncols) in enumerate(fchunks):
                    if not conv1_lookahead and ci > 0:
                        emit_conv1(b, X2, tt, c0, ncols)
                    hh = [hpool.tile([128, fch], f32r, tag="h",
                                     name=f"h{b}_{a}_{c0}") for a in range(4)]
                    la = conv1_lookahead and ci + 1 < len(fchunks)
                    if skip_combine:
                        continue
                    if nchains == 4:
                        emit_combine(b, tt, c0, ncols, (0, 1, 2, 3), hh)
                        if la:
                            emit_conv1(b, X2, tt, *fchunks[ci + 1])
                    else:
                        emit_combine(b, tt, c0, ncols, (0, 1), hh)
                        if la:
                            emit_conv1(b, X2, tt, *fchunks[ci + 1],
                                       irange=range(4))
                        emit_combine(b, tt, c0, ncols, (2, 3), hh)
                        if la:
                            emit_conv1(b, X2, tt, *fchunks[ci + 1],
                                       irange=range(4, 8))
                    if not skip_linear:
                        emit_linear_apply(b, tt, t8, h8, hh, c0, ncols)
                    # convT q-blocks whose frame window is fully masked
                    if ci == len(fchunks) - 1 and not skip_linear \
                            and not skip_combine:
                        for j in range(8):
                            nc.scalar.activation(tts[j][:, 0:NFP - 2],
                                                 tt[j][:, 1:NFP - 1], AF.Copy)
                    last = ci == len(fchunks) - 1
                    while not skip_convt and qb_done < NQB and (
                            last or (convt_interleave
                                     and 128 * qb_done + 131 <= c0 + ncols)):
                        emit_convT(b, tt, tts, t8, qb_done)
                        qb_done += 1
                if skip_convt:
                    emit_convT(b, tt, tts, t8, 0)
                if skip_combine and not skip_convt:
                    for j in range(8):
                        nc.scalar.activation(tts[j][:, 0:NFP - 2],
                                             tt[j][:, 1:NFP - 1], AF.Copy)
                    for qb in range(NQB):
                        emit_convT(b, tt, tts, t8, qb)

            def emit_all():
                ins = load_inputs(0)
                nxt = {}
                for b in range(BLOC):
                    if b + 1 < BLOC:
                        def prefetch(b=b):
                            nxt["ins"] = load_inputs(b + 1)
                        emit_batch(b, ins, prefetch)
                        ins = nxt.pop("ins")
                    else:
                        emit_batch(b, ins, None)

            if loop_reps == 1:
                emit_all()
            else:
                with tc.For_i(0, loop_reps, 1):
                    emit_all()
    nc.compile()
    return nc


_NC_CACHE = {}


def _get_nc(T, BLOC):
    key = (T, BLOC)
    if key not in _NC_CACHE:
        _NC_CACHE[key] = build_nc(T, BLOC)
    return _NC_CACHE[key]


def make_in_maps(x, conv1_w, in_gamma, comb1_w, comb1_b, comb2_w, comb2_b,
                 lin_w, lin_b, fc_gamma, convT_w):
    x = np.asarray(x)
    B, _, T = x.shape
    BLOC = B // N_CORES
    P = T // 128
    NF = P // 2 + 3
    NFP = NF + (NF & 1)
    w = pack_weights(conv1_w, in_gamma, comb1_w, comb1_b, comb2_w, comb2_b,
                     lin_w, lin_b, fc_gamma, convT_w)
    in_maps = []
    for core in range(N_CORES):
        shard = x[core * BLOC:(core + 1) * BLOC, 0, :]
        xt = np.ascontiguousarray(
            shard.reshape(BLOC, P, 128).transpose(0, 2, 1))
        t8, h8 = host_t8_h8(np.ascontiguousarray(shard), conv1_w, in_gamma,
                            comb1_w, comb1_b, comb2_w, comb2_b, NFP)
        m = {"x": xt, "t8in": t8, "h8in": h8}
        m.update(w)
        in_maps.append(m)
    return in_maps


def kernel(x, conv1_w, in_gamma, comb1_w, comb1_b, comb2_w, comb2_b,
           lin_w, lin_b, fc_gamma, convT_w):
    x = np.asarray(x)
    B, _, T = x.shape
    BLOC = B // N_CORES
    nc = _get_nc(T, BLOC)
    in_maps = make_in_maps(x, conv1_w, in_gamma, comb1_w, comb1_b, comb2_w,
                           comb2_b, lin_w, lin_b, fc_gamma, convT_w)
    res = run_bass_kernel_spmd(nc, in_maps, core_ids=list(range(N_CORES)))
    outs = [r["y"].reshape(BLOC, 2, T) for r in res.results]
    return np.concatenate(outs, axis=0)

